# revision 21
# baseline (speedup 1.0000x reference)
"""BitLinear (ternary weight quantization + linear) on 8 TRN2 NeuronCores.

y = x @ w_eff.T with w_eff = clip(round(w/scale), -1, 1) * scale,
scale = clamp(mean |w| per row, 1e-5).

The quantized weight is ternary, so the matmul is
y[m,o] = scale_o * sum_k q[o,k] * x[m,k] with q in {-1,0,1} -- exactly
representable in fp8e4. The PE's fp8 DoubleRow perf mode packs 2
contraction slots per cell (d = w0*m0 + w1*m1) at the same
columns-per-cycle rate as bf16, i.e. 2x the contraction throughput.

Slot budget: full-precision x needs a hi+lo e4m3 pair per k (no win over
bf16), while single e4m3 x measures ~1.8e-2 absmax error -- too close to
the 2e-2 gate. Compromise: x_hi = e4m3(bf16(x)) for all k, plus an
x_lo = e4m3(bf16(x) - x_hi) correction for half the k (k < 1024).
Measured 1.4e-2 absmax vs fp64, and 12 DoubleRow slots per 2048-k group
vs bf16's 16 -> 1.33x less PE matmul time. Products are exact in fp8
(ternary q), accumulation in fp32 PSUM.

Per accumulation group (one 512-wide out slice, K=2048): 12 DoubleRow
matmuls from ONE contiguous (x_t, w_all) tile pair -- 8 hi pairing
(k, k+1024), then 4 lo pairing (k, k+512) over k<1024. Keeping each
group's operands in single tiles at consecutive offsets matters: with
split hi/lo tiles the LDWEIGHTS stopped hiding under the matmul stream
and every matmul paid ~+106 cycles (259 ns vs 215 ns measured).

Sharding: 2 row-groups x 4 out-groups. Each core: x rows r*4096.. vs w
rows c*2048.. Per-core DMA 32 MiB x + 16 MiB w + 16 MiB y(fp16) = 64 MiB,
well under PE time. Per-row quantization is shard-local; the per-out-row
scale is applied at eviction via so_full (scales broadcast across
partitions once with a tiny f32r selector-matmul).

Schedule (merged fill pipeline): per step one W chunk advances (its
DMA+ternarize runs two steps ahead of its PE transposes), one x tile
stages, and up to two matmul groups run as their (x_t, w slice) pair
becomes ready; this keeps DMA (x+w+y) and PE (transposes+matmuls) both
loaded through the fill. Phase B then runs PE-bound -- 4 groups per
m-tile plus one leftover group per step -- with zero measured PE gaps.
"""

import numpy as np

import concourse.bass as bass
import concourse.mybir as mybir
import concourse.tile as tile
from concourse import bacc
from concourse.bass_utils import run_bass_kernel_spmd
from concourse.masks import make_identity

F32 = mybir.dt.float32
F32R = mybir.dt.float32r
BF16 = mybir.dt.bfloat16
F16 = mybir.dt.float16
F8 = mybir.dt.float8e4
DR = mybir.MatmulPerfMode.DoubleRow

# Problem shape (hardcoded per contract)
B, S, D_IN, D_OUT = 4, 2048, 2048, 8192
NCORES = 8
RGRP, CGRP = 2, 4          # core grid: row-groups x out-groups
R = B * S                  # 8192 rows of x
R_SH = R // RGRP           # 4096 rows per core
O_SH = D_OUT // CGRP       # 2048 out features per core
K_SUB = D_IN // 128        # 16 contraction sub-tiles
M_TILES = R_SH // 128      # 32 row tiles
O_TILES = O_SH // 128      # 16 weight row-tiles per core
N_SLICE = 512              # psum bank width (fp32)
N_SLICES = O_SH // N_SLICE # 4
N_HI = 8                   # hi DoubleRow matmuls per group (all 2048 k)
N_LO = 4                   # lo matmuls per group (k < 1024 corrected)
NPRE = 8                   # m-tiles that run n=0,1 only while W fills


def _build():
    nc = bacc.Bacc(None, target_bir_lowering=False)

    x_d = nc.dram_tensor("x", [R_SH, D_IN], F32, kind="ExternalInput")
    w_d = nc.dram_tensor("w", [O_SH, D_IN], F32, kind="ExternalInput")
    y_d = nc.dram_tensor("y", [R_SH, O_SH], F16, kind="ExternalOutput")

    with tile.TileContext(nc) as tc:
        with (
            tc.tile_pool(name="const", bufs=1) as const,
            tc.tile_pool(name="wt", bufs=1) as wtp,
            tc.tile_pool(name="ws", bufs=1) as ws,
            tc.tile_pool(name="xs", bufs=1) as xs,
            tc.tile_pool(name="ys", bufs=1) as ysp,
            tc.tile_pool(name="tp", bufs=1, space="PSUM") as tp,
            tc.tile_pool(name="ac", bufs=1, space="PSUM") as ac,
        ):
            ident_f = const.tile([128, 128], F32)
            make_identity(nc, ident_f[:])
            ident_bf = const.tile([128, 128], BF16)
            nc.vector.tensor_copy(ident_bf[:], ident_f[:])
            ident_fr = const.tile([128, 128], F32R)
            nc.vector.tensor_copy(ident_fr[:], ident_f[:])
            # sel[k, t*128+p] = (k==t): row-selector for the so broadcast
            sel_f = const.tile([4, 512], F32)
            nc.gpsimd.memset(sel_f[:], 0.0)
            nc.gpsimd.affine_select(
                out=sel_f[:].rearrange("p (t j) -> p t j", t=4),
                in_=sel_f[:].rearrange("p (t j) -> p t j", t=4),
                compare_op=mybir.AluOpType.not_equal,
                fill=1.0,
                base=0,
                pattern=[[-1, 4], [0, 128]],
                channel_multiplier=1,
            )
            sel = const.tile([4, 512], F32R)
            nc.vector.tensor_copy(sel[:], sel_f[:])

            # DoubleRow weight layout, resident in SBUF (fp8), one tile per
            # n-slice so each 12-matmul group streams consecutive offsets:
            # i in 0..7 (hi):  slot s holds q^T[s*1024 + i*128 + ki]
            # i in 8..11 (lo): slot s holds q^T[s*512 + (i-8)*128 + ki]
            w_all = [
                wtp.tile([128, N_HI + N_LO, 2, N_SLICE], F8, name=f"wal{n}")
                for n in range(N_SLICES)
            ]
            # so_full[n][p, o'] = scale of out column n*512+o' (any p)
            so_full = [
                wtp.tile([128, N_SLICE], F32, name=f"so{n}")
                for n in range(N_SLICES)
            ]
            so_col = wtp.tile([128, O_TILES], F32R, name="so_col")

            def w_quant(a):
                """DMA + quantize weight rows a*128..(a+1)*128 to ternary."""
                w_in = ws.tile([128, D_IN], F32, tag="w_in", bufs=2,
                               name=f"w_in_{a}")
                nc.sync.dma_start(w_in[:], w_d[a * 128 : (a + 1) * 128, :])

                scr = ws.tile([128, D_IN], F32, tag="w_scr", name=f"scr_{a}")
                ssum = ws.tile([128, 1], F32, tag="w_sum", name=f"ssum_{a}")
                nc.scalar.activation(
                    scr[:], w_in[:],
                    mybir.ActivationFunctionType.Abs,
                    accum_out=ssum[:],
                )
                scale = ws.tile([128, 1], F32, tag="w_scale",
                                name=f"scale_{a}")
                nc.vector.tensor_scalar(
                    out=scale[:], in0=ssum[:], scalar1=1.0 / D_IN,
                    scalar2=1e-5, op0=mybir.AluOpType.mult,
                    op1=mybir.AluOpType.max,
                )
                nc.vector.tensor_copy(so_col[:, a : a + 1], scale[:])
                hpos = ws.tile([128, 1], F32, tag="w_hpos", name=f"hp_{a}")
                hneg = ws.tile([128, 1], F32, tag="w_hneg", name=f"hn_{a}")
                nc.vector.tensor_scalar_mul(hpos[:], scale[:], 0.5)
                nc.vector.tensor_scalar_mul(hneg[:], scale[:], -0.5)

                # q = (w > 0.5*scale) - (w < -0.5*scale) in bf16 (exact)
                # (strict > matches round-half-even of round(w/s) at 0.5)
                qp = ws.tile([128, D_IN], BF16, tag="w_qp", name=f"qp_{a}")
                nc.vector.tensor_scalar(
                    out=qp[:], in0=w_in[:], scalar1=hpos[:], scalar2=None,
                    op0=mybir.AluOpType.is_gt,
                )
                qn = ws.tile([128, D_IN], BF16, tag="w_qn", name=f"qn_{a}")
                nc.vector.tensor_scalar(
                    out=qn[:], in0=w_in[:], scalar1=hneg[:], scalar2=None,
                    op0=mybir.AluOpType.is_lt,
                )
                q = ws.tile([128, D_IN], BF16, tag="w_q", bufs=3,
                            name=f"q_{a}")
                nc.vector.tensor_sub(q[:], qp[:], qn[:])
                return q

            def w_emit(a, q):
                """Transpose ternary q and evict into DoubleRow layouts."""
                n_idx, o_off = divmod(a * 128, N_SLICE)
                for g in range(2):
                    wt_ps = tp.tile([128, 8, 128], BF16, tag="xtps", bufs=4,
                                    name=f"wpt_{a}_{g}")
                    for j in range(8):
                        k = g * 8 + j
                        nc.tensor.transpose(
                            wt_ps[:, j, :], q[:, k * 128 : (k + 1) * 128],
                            ident_bf[:],
                        )
                    osl = slice(o_off, o_off + 128)
                    # hi: k16 0..7 -> slot 0, k16 8..15 -> slot 1
                    nc.scalar.copy(
                        w_all[n_idx][:, 0:N_HI, g, osl], wt_ps[:]
                    )
                    if g == 0:
                        # lo: k16 0..3 -> slot 0, k16 4..7 -> slot 1
                        nc.vector.tensor_copy(
                            w_all[n_idx][:, N_HI : N_HI + N_LO, 0, osl],
                            wt_ps[:, 0:4, :],
                        )
                        nc.vector.tensor_copy(
                            w_all[n_idx][:, N_HI : N_HI + N_LO, 1, osl],
                            wt_ps[:, 4:8, :],
                        )

            def so_slice(n):
                """Broadcast scales of slice n across partitions."""
                soT_sb = ws.tile([4, 128], F32R, tag="soT", name=f"soT_{n}")
                t_ps = ac.tile([128, N_SLICE], F32, tag="acc", bufs=4,
                               name=f"sot_ps_{n}")
                nc.tensor.transpose(
                    t_ps[0:4, 0:128].bitcast(F32R),
                    so_col[:, 4 * n : 4 * n + 4],
                    ident_fr[:],
                )
                nc.scalar.copy(soT_sb[:], t_ps[0:4, 0:128])
                bc = ac.tile([128, N_SLICE], F32, tag="acc", bufs=4,
                             name=f"so_bc_{n}")
                for t in range(4):
                    nc.tensor.matmul(
                        bc[:, t * 128 : (t + 1) * 128],
                        sel[:, t * 128 : (t + 1) * 128],
                        soT_sb[:],
                        start=True, stop=True,
                    )
                nc.scalar.copy(so_full[n][:], bc[:])

            def x_prefetch(m):
                x_in = xs.tile([128, D_IN], F32, tag="x_in", bufs=4,
                               name=f"x_in_{m}")
                nc.sync.dma_start(x_in[:], x_d[m * 128 : (m + 1) * 128, :])
                return x_in

            def x_stage(m, x_in=None):
                """Load x row-tile m, bf16, transpose, hi/lo split to fp8."""
                if x_in is None:
                    x_in = x_prefetch(m)
                x_bf = xs.tile([128, D_IN], BF16, tag="x_bf", bufs=2,
                               name=f"x_bf_{m}")

                x_t = xs.tile([128, N_HI + N_LO, 2, 128], F8, tag="x_t",
                              bufs=14, name=f"x_t_{m}")
                for g in range(2):
                    gsl = slice(g * 1024, (g + 1) * 1024)
                    nc.scalar.copy(x_bf[:, gsl], x_in[:, gsl])
                    pt = tp.tile([128, 8, 128], BF16, tag="xtps", bufs=4,
                                 name=f"xpt_{m}_{g}")
                    for j in range(8):
                        k = g * 8 + j
                        nc.tensor.transpose(
                            pt[:, j, :], x_bf[:, k * 128 : (k + 1) * 128],
                            ident_bf[:],
                        )
                    hi = x_t[:, 0:N_HI, g, :]
                    nc.vector.tensor_copy(hi, pt[:])
                    if g == 0:
                        # lo = bf16(x) - hi for k < 1024, into (k, k+512)
                        nc.vector.tensor_tensor(
                            out=x_t[:, N_HI : N_HI + N_LO, 0, :],
                            in0=pt[:, 0:4, :], in1=hi[:, 0:4, :],
                            op=mybir.AluOpType.subtract,
                        )
                        nc.vector.tensor_tensor(
                            out=x_t[:, N_HI : N_HI + N_LO, 1, :],
                            in0=pt[:, 4:8, :], in1=hi[:, 4:8, :],
                            op=mybir.AluOpType.subtract,
                        )
                return x_t

            def mm_group(m, n, x_t):
                """One 12-matmul DoubleRow group + scaled fp16 store."""
                nmm = N_HI + N_LO
                acc = ac.tile([128, N_SLICE], F32, tag="acc", bufs=4,
                              name=f"acc{n}_{m}")
                for i in range(nmm):
                    nc.tensor.matmul(
                        acc[:],
                        x_t[:, i, :, :],
                        w_all[n][:, i, :, :],
                        start=(i == 0),
                        stop=(i == nmm - 1),
                        perf_mode=DR,
                    )
                y_sb = ysp.tile([128, N_SLICE], F16, tag="y_sb", bufs=6,
                                name=f"y_sb{n}_{m}")
                nc.vector.tensor_tensor(
                    out=y_sb[:], in0=acc[:], in1=so_full[n][:],
                    op=mybir.AluOpType.mult,
                )
                nc.sync.dma_start(
                    y_d[m * 128 : (m + 1) * 128,
                        n * N_SLICE : (n + 1) * N_SLICE],
                    y_sb[:],
                )

            # ---- schedule ----
            # Merged pipeline: per step, one W chunk advances (quant 2
            # ahead of its PE transposes), one x tile stages, and up to
            # two matmul groups run as their (x_t, w slice) pair becomes
            # ready. This keeps the DMA (x+w+y) and PE (transposes+mm)
            # both ~80-90% loaded through the fill; phase B then runs
            # PE-bound with the leftover groups drained one per step.
            xts = {}
            qs = {}
            ready_n = set()
            pend = []           # mm groups still to run for m < NPRE
            xin0 = x_prefetch(0)
            xin1 = x_prefetch(1)
            qs[0] = w_quant(0)
            qs[1] = w_quant(1)
            xts[0] = x_stage(0, xin0)
            xts[1] = x_stage(1, xin1)
            xq = list(range(2, NPRE + 2))   # x tiles to stage during fill

            def run_avail(budget):
                ran = 0
                for mn in list(pend):
                    if ran >= budget:
                        break
                    m, n = mn
                    if n in ready_n and m in xts:
                        mm_group(m, n, xts[m])
                        pend.remove(mn)
                        ran += 1
                return ran

            for s_ in range(O_TILES):
                w_emit(s_, qs.pop(s_))
                if s_ + 2 < O_TILES:
                    qs[s_ + 2] = w_quant(s_ + 2)
                if s_ % 4 == 1:
                    # so_slice(n) needs only the quant-time scales of
                    # chunks 4n..4n+3 (all emitted by step 4n+1); doing it
                    # here keeps later quants' so_col writes from
                    # serializing against the so transpose
                    so_slice(s_ // 4)
                if s_ % 4 == 3:
                    n = s_ // 4
                    ready_n.add(n)
                    pend.extend((m, n) for m in range(NPRE))
                for _ in range(2 if s_ < 2 else 1):
                    if xq:
                        mx = xq.pop(0)
                        xts[mx] = x_stage(mx)
                run_avail(3 if s_ >= 12 else 2)

            # Phase B: full groups for m>=NPRE plus backlog drain
            for m in range(NPRE, M_TILES):
                if m not in xts:
                    xts[m] = x_stage(m)
                for n in range(N_SLICES):
                    mm_group(m, n, xts[m])
                run_avail(1)
                for mp in list(range(NPRE)):
                    if mp in xts and not any(b[0] == mp for b in pend):
                        del xts[mp]
                if m + 2 < M_TILES and (m + 2) not in xts:
                    xts[m + 2] = x_stage(m + 2)
            while pend:
                run_avail(len(pend))

    nc.compile()
    return nc


_NC_CACHE = None


def _get_nc():
    global _NC_CACHE
    if _NC_CACHE is None:
        _NC_CACHE = _build()
    return _NC_CACHE


def kernel(x: np.ndarray, weight: np.ndarray, _trace: bool = False):
    assert x.shape == (B, S, D_IN) and weight.shape == (D_OUT, D_IN)
    x_flat = np.ascontiguousarray(x.reshape(R, D_IN), dtype=np.float32)
    in_maps = []
    for c in range(NCORES):
        r, col = divmod(c, CGRP)
        in_maps.append(
            {
                "x": np.ascontiguousarray(x_flat[r * R_SH : (r + 1) * R_SH]),
                "w": np.ascontiguousarray(
                    weight[col * O_SH : (col + 1) * O_SH], dtype=np.float32
                ),
            }
        )
    nc = _get_nc()
    res = run_bass_kernel_spmd(
        nc, in_maps, core_ids=list(range(NCORES)), trace=_trace
    )
    y = np.empty((R, D_OUT), dtype=np.float32)
    for c in range(NCORES):
        r, col = divmod(c, CGRP)
        y[r * R_SH : (r + 1) * R_SH, col * O_SH : (col + 1) * O_SH] = (
            res.results[c]["y"]
        )
    out = y.reshape(B, S, D_OUT)
    if _trace:
        return out, res
    return out


# revision 23
# speedup vs baseline: 1.0071x; 1.0071x over previous
"""BitLinear (ternary weight quantization + linear) on 8 TRN2 NeuronCores.

y = x @ w_eff.T with w_eff = clip(round(w/scale), -1, 1) * scale,
scale = clamp(mean |w| per row, 1e-5).

The quantized weight is ternary, so the matmul is
y[m,o] = scale_o * sum_k q[o,k] * x[m,k] with q in {-1,0,1} -- exactly
representable in fp8e4. The PE's fp8 DoubleRow perf mode packs 2
contraction slots per cell (d = w0*m0 + w1*m1) at the same
columns-per-cycle rate as bf16, i.e. 2x the contraction throughput.

Slot budget: full-precision x needs a hi+lo e4m3 pair per k (no win over
bf16), while single e4m3 x measures ~1.8e-2 absmax error -- too close to
the 2e-2 gate. Compromise: x_hi = e4m3(bf16(x)) for all k, plus an
x_lo = e4m3(bf16(x) - x_hi) correction for half the k (k < 1024).
Measured 1.4e-2 absmax vs fp64, and 12 DoubleRow slots per 2048-k group
vs bf16's 16 -> 1.33x less PE matmul time. Products are exact in fp8
(ternary q), accumulation in fp32 PSUM.

Per accumulation group (one 512-wide out slice, K=2048): 12 DoubleRow
matmuls from ONE contiguous (x_t, w_all) tile pair -- 8 hi pairing
(k, k+1024), then 4 lo pairing (k, k+512) over k<1024. Keeping each
group's operands in single tiles at consecutive offsets matters: with
split hi/lo tiles the LDWEIGHTS stopped hiding under the matmul stream
and every matmul paid ~+106 cycles (259 ns vs 215 ns measured).

Sharding: 2 row-groups x 4 out-groups. Each core: x rows r*4096.. vs w
rows c*2048.. Per-core DMA 32 MiB x + 16 MiB w + 16 MiB y(fp16) = 64 MiB,
well under PE time. Per-row quantization is shard-local; the per-out-row
scale is applied at eviction via so_full (scales broadcast across
partitions once with a tiny f32r selector-matmul).

Schedule (merged fill pipeline): per step one W chunk advances (its
DMA+ternarize runs two steps ahead of its PE transposes), one x tile
stages, and up to two matmul groups run as their (x_t, w slice) pair
becomes ready; this keeps DMA (x+w+y) and PE (transposes+matmuls) both
loaded through the fill. Phase B then runs PE-bound -- 4 groups per
m-tile plus one leftover group per step -- with zero measured PE gaps.
"""

import numpy as np

import concourse.bass as bass
import concourse.mybir as mybir
import concourse.tile as tile
from concourse import bacc
from concourse.bass_utils import run_bass_kernel_spmd
from concourse.masks import make_identity

F32 = mybir.dt.float32
F32R = mybir.dt.float32r
BF16 = mybir.dt.bfloat16
F16 = mybir.dt.float16
F8 = mybir.dt.float8e4
DR = mybir.MatmulPerfMode.DoubleRow

# Problem shape (hardcoded per contract)
B, S, D_IN, D_OUT = 4, 2048, 2048, 8192
NCORES = 8
RGRP, CGRP = 2, 4          # core grid: row-groups x out-groups
R = B * S                  # 8192 rows of x
R_SH = R // RGRP           # 4096 rows per core
O_SH = D_OUT // CGRP       # 2048 out features per core
K_SUB = D_IN // 128        # 16 contraction sub-tiles
M_TILES = R_SH // 128      # 32 row tiles
O_TILES = O_SH // 128      # 16 weight row-tiles per core
N_SLICE = 512              # psum bank width (fp32)
N_SLICES = O_SH // N_SLICE # 4
N_HI = 8                   # hi DoubleRow matmuls per group (all 2048 k)
N_LO = 4                   # lo matmuls per group (k < 1024 corrected)
NPRE = 8                   # m-tiles that run n=0,1 only while W fills


def _build():
    nc = bacc.Bacc(None, target_bir_lowering=False)

    x_d = nc.dram_tensor("x", [R_SH, D_IN], F32, kind="ExternalInput")
    w_d = nc.dram_tensor("w", [O_SH, D_IN], F32, kind="ExternalInput")
    y_d = nc.dram_tensor("y", [R_SH, O_SH], F16, kind="ExternalOutput")

    with tile.TileContext(nc) as tc:
        with (
            tc.tile_pool(name="const", bufs=1) as const,
            tc.tile_pool(name="wt", bufs=1) as wtp,
            tc.tile_pool(name="ws", bufs=1) as ws,
            tc.tile_pool(name="xs", bufs=1) as xs,
            tc.tile_pool(name="ys", bufs=1) as ysp,
            tc.tile_pool(name="tp", bufs=1, space="PSUM") as tp,
            tc.tile_pool(name="ac", bufs=1, space="PSUM") as ac,
        ):
            ident_f = const.tile([128, 128], F32)
            make_identity(nc, ident_f[:])
            ident_bf = const.tile([128, 128], BF16)
            nc.vector.tensor_copy(ident_bf[:], ident_f[:])
            ident_fr = const.tile([128, 128], F32R)
            nc.vector.tensor_copy(ident_fr[:], ident_f[:])
            # sel[k, t*128+p] = (k==t): row-selector for the so broadcast
            sel_f = const.tile([4, 512], F32)
            nc.gpsimd.memset(sel_f[:], 0.0)
            nc.gpsimd.affine_select(
                out=sel_f[:].rearrange("p (t j) -> p t j", t=4),
                in_=sel_f[:].rearrange("p (t j) -> p t j", t=4),
                compare_op=mybir.AluOpType.not_equal,
                fill=1.0,
                base=0,
                pattern=[[-1, 4], [0, 128]],
                channel_multiplier=1,
            )
            sel = const.tile([4, 512], F32R)
            nc.vector.tensor_copy(sel[:], sel_f[:])

            # DoubleRow weight layout, resident in SBUF (fp8), one tile per
            # n-slice so each 12-matmul group streams consecutive offsets:
            # i in 0..7 (hi):  slot s holds q^T[s*1024 + i*128 + ki]
            # i in 8..11 (lo): slot s holds q^T[s*512 + (i-8)*128 + ki]
            w_all = [
                wtp.tile([128, N_HI + N_LO, 2, N_SLICE], F8, name=f"wal{n}")
                for n in range(N_SLICES)
            ]
            # so_full[n][p, o'] = scale of out column n*512+o' (any p)
            so_full = [
                wtp.tile([128, N_SLICE], F32, name=f"so{n}")
                for n in range(N_SLICES)
            ]
            so_col = wtp.tile([128, O_TILES], F32R, name="so_col")

            def w_quant(a):
                """DMA + quantize weight rows a*128..(a+1)*128 to ternary."""
                w_in = ws.tile([128, D_IN], F32, tag="w_in", bufs=2,
                               name=f"w_in_{a}")
                nc.sync.dma_start(w_in[:], w_d[a * 128 : (a + 1) * 128, :])

                scr = ws.tile([128, D_IN], F32, tag="w_scr", name=f"scr_{a}")
                ssum = ws.tile([128, 1], F32, tag="w_sum", name=f"ssum_{a}")
                nc.scalar.activation(
                    scr[:], w_in[:],
                    mybir.ActivationFunctionType.Abs,
                    accum_out=ssum[:],
                )
                scale = ws.tile([128, 1], F32, tag="w_scale",
                                name=f"scale_{a}")
                nc.vector.tensor_scalar(
                    out=scale[:], in0=ssum[:], scalar1=1.0 / D_IN,
                    scalar2=1e-5, op0=mybir.AluOpType.mult,
                    op1=mybir.AluOpType.max,
                )
                nc.vector.tensor_copy(so_col[:, a : a + 1], scale[:])
                hpos = ws.tile([128, 1], F32, tag="w_hpos", name=f"hp_{a}")
                hneg = ws.tile([128, 1], F32, tag="w_hneg", name=f"hn_{a}")
                nc.vector.tensor_scalar_mul(hpos[:], scale[:], 0.5)
                nc.vector.tensor_scalar_mul(hneg[:], scale[:], -0.5)

                # q = (w > 0.5*scale) - (w < -0.5*scale) in bf16 (exact)
                # (strict > matches round-half-even of round(w/s) at 0.5)
                qp = ws.tile([128, D_IN], BF16, tag="w_qp", name=f"qp_{a}")
                nc.vector.tensor_scalar(
                    out=qp[:], in0=w_in[:], scalar1=hpos[:], scalar2=None,
                    op0=mybir.AluOpType.is_gt,
                )
                qn = ws.tile([128, D_IN], BF16, tag="w_qn", name=f"qn_{a}")
                nc.vector.tensor_scalar(
                    out=qn[:], in0=w_in[:], scalar1=hneg[:], scalar2=None,
                    op0=mybir.AluOpType.is_lt,
                )
                q = ws.tile([128, D_IN], BF16, tag="w_q", bufs=3,
                            name=f"q_{a}")
                nc.vector.tensor_sub(q[:], qp[:], qn[:])
                return q

            def w_emit(a, q):
                """Transpose ternary q and evict into DoubleRow layouts."""
                n_idx, o_off = divmod(a * 128, N_SLICE)
                for g in range(2):
                    wt_ps = tp.tile([128, 8, 128], BF16, tag="xtps", bufs=4,
                                    name=f"wpt_{a}_{g}")
                    for j in range(8):
                        k = g * 8 + j
                        nc.tensor.transpose(
                            wt_ps[:, j, :], q[:, k * 128 : (k + 1) * 128],
                            ident_bf[:],
                        )
                    osl = slice(o_off, o_off + 128)
                    # hi: k16 0..7 -> slot 0, k16 8..15 -> slot 1
                    nc.scalar.copy(
                        w_all[n_idx][:, 0:N_HI, g, osl], wt_ps[:]
                    )
                    if g == 0:
                        # lo: k16 0..3 -> slot 0, k16 4..7 -> slot 1
                        nc.vector.tensor_copy(
                            w_all[n_idx][:, N_HI : N_HI + N_LO, 0, osl],
                            wt_ps[:, 0:4, :],
                        )
                        nc.vector.tensor_copy(
                            w_all[n_idx][:, N_HI : N_HI + N_LO, 1, osl],
                            wt_ps[:, 4:8, :],
                        )

            def so_slice(n):
                """Broadcast scales of slice n across partitions."""
                soT_sb = ws.tile([4, 128], F32R, tag="soT", name=f"soT_{n}")
                t_ps = ac.tile([128, N_SLICE], F32, tag="acc", bufs=4,
                               name=f"sot_ps_{n}")
                nc.tensor.transpose(
                    t_ps[0:4, 0:128].bitcast(F32R),
                    so_col[:, 4 * n : 4 * n + 4],
                    ident_fr[:],
                )
                nc.scalar.copy(soT_sb[:], t_ps[0:4, 0:128])
                bc = ac.tile([128, N_SLICE], F32, tag="acc", bufs=4,
                             name=f"so_bc_{n}")
                for t in range(4):
                    nc.tensor.matmul(
                        bc[:, t * 128 : (t + 1) * 128],
                        sel[:, t * 128 : (t + 1) * 128],
                        soT_sb[:],
                        start=True, stop=True,
                    )
                nc.scalar.copy(so_full[n][:], bc[:])

            def x_prefetch(m):
                x_in = xs.tile([128, D_IN], F32, tag="x_in", bufs=4,
                               name=f"x_in_{m}")
                nc.sync.dma_start(x_in[:], x_d[m * 128 : (m + 1) * 128, :])
                return x_in

            def x_stage(m, x_in=None):
                """Load x row-tile m, bf16, transpose, hi/lo split to fp8."""
                if x_in is None:
                    x_in = x_prefetch(m)
                x_bf = xs.tile([128, D_IN], BF16, tag="x_bf", bufs=2,
                               name=f"x_bf_{m}")

                x_t = xs.tile([128, N_HI + N_LO, 2, 128], F8, tag="x_t",
                              bufs=14, name=f"x_t_{m}")
                for g in range(2):
                    gsl = slice(g * 1024, (g + 1) * 1024)
                    nc.scalar.copy(x_bf[:, gsl], x_in[:, gsl])
                    pt = tp.tile([128, 8, 128], BF16, tag="xtps", bufs=4,
                                 name=f"xpt_{m}_{g}")
                    for j in range(8):
                        k = g * 8 + j
                        nc.tensor.transpose(
                            pt[:, j, :], x_bf[:, k * 128 : (k + 1) * 128],
                            ident_bf[:],
                        )
                    hi = x_t[:, 0:N_HI, g, :]
                    nc.vector.tensor_copy(hi, pt[:])
                    if g == 0:
                        # lo = bf16(x) - hi for k < 1024, into (k, k+512)
                        nc.vector.tensor_tensor(
                            out=x_t[:, N_HI : N_HI + N_LO, 0, :],
                            in0=pt[:, 0:4, :], in1=hi[:, 0:4, :],
                            op=mybir.AluOpType.subtract,
                        )
                        nc.vector.tensor_tensor(
                            out=x_t[:, N_HI : N_HI + N_LO, 1, :],
                            in0=pt[:, 4:8, :], in1=hi[:, 4:8, :],
                            op=mybir.AluOpType.subtract,
                        )
                return x_t

            def mm_group(m, n, x_t):
                """One 12-matmul DoubleRow group + scaled fp16 store."""
                nmm = N_HI + N_LO
                acc = ac.tile([128, N_SLICE], F32, tag="acc", bufs=4,
                              name=f"acc{n}_{m}")
                for i in range(nmm):
                    nc.tensor.matmul(
                        acc[:],
                        x_t[:, i, :, :],
                        w_all[n][:, i, :, :],
                        start=(i == 0),
                        stop=(i == nmm - 1),
                        perf_mode=DR,
                    )
                y_sb = ysp.tile([128, N_SLICE], F16, tag="y_sb", bufs=6,
                                name=f"y_sb{n}_{m}")
                nc.vector.tensor_tensor(
                    out=y_sb[:], in0=acc[:], in1=so_full[n][:],
                    op=mybir.AluOpType.mult,
                )
                nc.sync.dma_start(
                    y_d[m * 128 : (m + 1) * 128,
                        n * N_SLICE : (n + 1) * N_SLICE],
                    y_sb[:],
                )

            # ---- schedule ----
            # Merged pipeline: per step, one W chunk advances (quant 2
            # ahead of its PE transposes), one x tile stages, and up to
            # two matmul groups run as their (x_t, w slice) pair becomes
            # ready. This keeps the DMA (x+w+y) and PE (transposes+mm)
            # both ~80-90% loaded through the fill; phase B then runs
            # PE-bound with the leftover groups drained one per step.
            xts = {}
            qs = {}
            ready_n = set()
            pend = []           # mm groups still to run for m < NPRE
            xin0 = x_prefetch(0)
            xin1 = x_prefetch(1)
            qs[0] = w_quant(0)
            qs[1] = w_quant(1)
            xts[0] = x_stage(0, xin0)
            xts[1] = x_stage(1, xin1)
            xq = list(range(2, NPRE + 2))   # x tiles to stage during fill

            def run_avail(budget):
                ran = 0
                for mn in list(pend):
                    if ran >= budget:
                        break
                    m, n = mn
                    if n in ready_n and m in xts:
                        mm_group(m, n, xts[m])
                        pend.remove(mn)
                        ran += 1
                return ran

            for s_ in range(O_TILES):
                # evictions (w transposes, x stages, mm groups) first so
                # the DVE drains the PSUM transpose tiles needed by the
                # next step before chewing the 4.3us quant chain, which
                # isn't needed until two steps later
                w_emit(s_, qs.pop(s_))
                if s_ % 4 == 3:
                    so_slice(s_ // 4)
                    n = s_ // 4
                    ready_n.add(n)
                    pend.extend((m, n) for m in range(NPRE))
                for _ in range(2 if s_ < 2 else 1):
                    if xq:
                        mx = xq.pop(0)
                        xts[mx] = x_stage(mx)
                run_avail(2)
                if s_ + 2 < O_TILES:
                    qs[s_ + 2] = w_quant(s_ + 2)

            # Phase B: full groups for m>=NPRE plus backlog drain
            for m in range(NPRE, M_TILES):
                if m not in xts:
                    xts[m] = x_stage(m)
                for n in range(N_SLICES):
                    mm_group(m, n, xts[m])
                run_avail(1)
                for mp in list(range(NPRE)):
                    if mp in xts and not any(b[0] == mp for b in pend):
                        del xts[mp]
                if m + 2 < M_TILES and (m + 2) not in xts:
                    xts[m + 2] = x_stage(m + 2)
            while pend:
                run_avail(len(pend))

    nc.compile()
    return nc


_NC_CACHE = None


def _get_nc():
    global _NC_CACHE
    if _NC_CACHE is None:
        _NC_CACHE = _build()
    return _NC_CACHE


def kernel(x: np.ndarray, weight: np.ndarray, _trace: bool = False):
    assert x.shape == (B, S, D_IN) and weight.shape == (D_OUT, D_IN)
    x_flat = np.ascontiguousarray(x.reshape(R, D_IN), dtype=np.float32)
    in_maps = []
    for c in range(NCORES):
        r, col = divmod(c, CGRP)
        in_maps.append(
            {
                "x": np.ascontiguousarray(x_flat[r * R_SH : (r + 1) * R_SH]),
                "w": np.ascontiguousarray(
                    weight[col * O_SH : (col + 1) * O_SH], dtype=np.float32
                ),
            }
        )
    nc = _get_nc()
    res = run_bass_kernel_spmd(
        nc, in_maps, core_ids=list(range(NCORES)), trace=_trace
    )
    y = np.empty((R, D_OUT), dtype=np.float32)
    for c in range(NCORES):
        r, col = divmod(c, CGRP)
        y[r * R_SH : (r + 1) * R_SH, col * O_SH : (col + 1) * O_SH] = (
            res.results[c]["y"]
        )
    out = y.reshape(B, S, D_OUT)
    if _trace:
        return out, res
    return out


# revision 24
# speedup vs baseline: 1.0447x; 1.0374x over previous
"""BitLinear (ternary weight quantization + linear) on 8 TRN2 NeuronCores.

y = x @ w_eff.T with w_eff = clip(round(w/scale), -1, 1) * scale,
scale = clamp(mean |w| per row, 1e-5).

The quantized weight is ternary, so the matmul is
y[m,o] = scale_o * sum_k q[o,k] * x[m,k] with q in {-1,0,1} -- exactly
representable in fp8e4. The PE's fp8 DoubleRow perf mode packs 2
contraction slots per cell (d = w0*m0 + w1*m1) at the same
columns-per-cycle rate as bf16, i.e. 2x the contraction throughput.

Slot budget: full-precision x needs a hi+lo e4m3 pair per k (no win over
bf16), while single e4m3 x measures ~1.8e-2 absmax error -- too close to
the 2e-2 gate. Compromise: x_hi = e4m3(bf16(x)) for all k, plus an
x_lo = e4m3(bf16(x) - x_hi) correction for half the k (k < 1024).
Measured 1.4e-2 absmax vs fp64, and 12 DoubleRow slots per 2048-k group
vs bf16's 16 -> 1.33x less PE matmul time. Products are exact in fp8
(ternary q), accumulation in fp32 PSUM.

Per accumulation group (one 512-wide out slice, K=2048): 12 DoubleRow
matmuls from ONE contiguous (x_t, w_all) tile pair -- 8 hi pairing
(k, k+1024), then 4 lo pairing (k, k+512) over k<1024. Keeping each
group's operands in single tiles at consecutive offsets matters: with
split hi/lo tiles the LDWEIGHTS stopped hiding under the matmul stream
and every matmul paid ~+106 cycles (259 ns vs 215 ns measured).

Sharding: 2 row-groups x 4 out-groups. Each core: x rows r*4096.. vs w
rows c*2048.. Per-core DMA 32 MiB x + 16 MiB w + 16 MiB y(fp16) = 64 MiB,
well under PE time. Per-row quantization is shard-local; the per-out-row
scale is applied at eviction via so_full (scales broadcast across
partitions once with a tiny f32r selector-matmul).

Schedule (merged fill pipeline): per step one W chunk advances (its
DMA+ternarize runs two steps ahead of its PE transposes), one x tile
stages, and up to two matmul groups run as their (x_t, w slice) pair
becomes ready; this keeps DMA (x+w+y) and PE (transposes+matmuls) both
loaded through the fill. Phase B then runs PE-bound -- 4 groups per
m-tile plus one leftover group per step -- with zero measured PE gaps.
"""

import numpy as np

import concourse.bass as bass
import concourse.mybir as mybir
import concourse.tile as tile
from concourse import bacc
from concourse.bass_utils import run_bass_kernel_spmd
from concourse.masks import make_identity

F32 = mybir.dt.float32
F32R = mybir.dt.float32r
BF16 = mybir.dt.bfloat16
F16 = mybir.dt.float16
F8 = mybir.dt.float8e4
DR = mybir.MatmulPerfMode.DoubleRow

# Problem shape (hardcoded per contract)
B, S, D_IN, D_OUT = 4, 2048, 2048, 8192
NCORES = 8
RGRP, CGRP = 2, 4          # core grid: row-groups x out-groups
R = B * S                  # 8192 rows of x
R_SH = R // RGRP           # 4096 rows per core
O_SH = D_OUT // CGRP       # 2048 out features per core
K_SUB = D_IN // 128        # 16 contraction sub-tiles
M_TILES = R_SH // 128      # 32 row tiles
O_TILES = O_SH // 128      # 16 weight row-tiles per core
N_SLICE = 512              # psum bank width (fp32)
N_SLICES = O_SH // N_SLICE # 4
N_HI = 8                   # hi DoubleRow matmuls per group (all 2048 k)
N_LO = 4                   # lo matmuls per group (k < 1024 corrected)
NPRE = 8                   # m-tiles that run n=0,1 only while W fills


def _build():
    nc = bacc.Bacc(None, target_bir_lowering=False)

    x_d = nc.dram_tensor("x", [R_SH, D_IN], F32, kind="ExternalInput")
    w_d = nc.dram_tensor("w", [O_SH, D_IN], F32, kind="ExternalInput")
    y_d = nc.dram_tensor("y", [R_SH, O_SH], F16, kind="ExternalOutput")

    with tile.TileContext(nc) as tc:
        with (
            tc.tile_pool(name="const", bufs=1) as const,
            tc.tile_pool(name="wt", bufs=1) as wtp,
            tc.tile_pool(name="ws", bufs=1) as ws,
            tc.tile_pool(name="xs", bufs=1) as xs,
            tc.tile_pool(name="ys", bufs=1) as ysp,
            tc.tile_pool(name="tp", bufs=1, space="PSUM") as tp,
            tc.tile_pool(name="ac", bufs=1, space="PSUM") as ac,
        ):
            ident_f = const.tile([128, 128], F32)
            make_identity(nc, ident_f[:])
            ident_bf = const.tile([128, 128], BF16)
            nc.vector.tensor_copy(ident_bf[:], ident_f[:])
            ident_fr = const.tile([128, 128], F32R)
            nc.vector.tensor_copy(ident_fr[:], ident_f[:])
            # sel[k, t*128+p] = (k==t): row-selector for the so broadcast
            sel_f = const.tile([4, 512], F32)
            nc.gpsimd.memset(sel_f[:], 0.0)
            nc.gpsimd.affine_select(
                out=sel_f[:].rearrange("p (t j) -> p t j", t=4),
                in_=sel_f[:].rearrange("p (t j) -> p t j", t=4),
                compare_op=mybir.AluOpType.not_equal,
                fill=1.0,
                base=0,
                pattern=[[-1, 4], [0, 128]],
                channel_multiplier=1,
            )
            sel = const.tile([4, 512], F32R)
            nc.vector.tensor_copy(sel[:], sel_f[:])

            # DoubleRow weight layout, resident in SBUF (fp8), one tile per
            # n-slice so each 12-matmul group streams consecutive offsets:
            # i in 0..7 (hi):  slot s holds q^T[s*1024 + i*128 + ki]
            # i in 8..11 (lo): slot s holds q^T[s*512 + (i-8)*128 + ki]
            w_all = [
                wtp.tile([128, N_HI + N_LO, 2, N_SLICE], F8, name=f"wal{n}")
                for n in range(N_SLICES)
            ]
            # so_full[n][p, o'] = scale of out column n*512+o' (any p)
            so_full = [
                wtp.tile([128, N_SLICE], F32, name=f"so{n}")
                for n in range(N_SLICES)
            ]
            so_col = wtp.tile([128, O_TILES], F32R, name="so_col")

            def w_quant(a):
                """DMA + quantize weight rows a*128..(a+1)*128 to ternary."""
                w_in = ws.tile([128, D_IN], F32, tag="w_in", bufs=2,
                               name=f"w_in_{a}")
                nc.sync.dma_start(w_in[:], w_d[a * 128 : (a + 1) * 128, :])

                scr = ws.tile([128, D_IN], F32, tag="w_scr", name=f"scr_{a}")
                ssum = ws.tile([128, 1], F32, tag="w_sum", name=f"ssum_{a}")
                nc.scalar.activation(
                    scr[:], w_in[:],
                    mybir.ActivationFunctionType.Abs,
                    accum_out=ssum[:],
                )
                scale = ws.tile([128, 1], F32, tag="w_scale",
                                name=f"scale_{a}")
                nc.vector.tensor_scalar(
                    out=scale[:], in0=ssum[:], scalar1=1.0 / D_IN,
                    scalar2=1e-5, op0=mybir.AluOpType.mult,
                    op1=mybir.AluOpType.max,
                )
                nc.vector.tensor_copy(so_col[:, a : a + 1], scale[:])
                hpos = ws.tile([128, 1], F32, tag="w_hpos", name=f"hp_{a}")
                hneg = ws.tile([128, 1], F32, tag="w_hneg", name=f"hn_{a}")
                nc.vector.tensor_scalar_mul(hpos[:], scale[:], 0.5)
                nc.vector.tensor_scalar_mul(hneg[:], scale[:], -0.5)

                # q = (w > 0.5*scale) - (w < -0.5*scale) in bf16 (exact)
                # (strict > matches round-half-even of round(w/s) at 0.5)
                qp = ws.tile([128, D_IN], BF16, tag="w_qp", name=f"qp_{a}")
                nc.vector.tensor_scalar(
                    out=qp[:], in0=w_in[:], scalar1=hpos[:], scalar2=None,
                    op0=mybir.AluOpType.is_gt,
                )
                qn = ws.tile([128, D_IN], BF16, tag="w_qn", name=f"qn_{a}")
                nc.vector.tensor_scalar(
                    out=qn[:], in0=w_in[:], scalar1=hneg[:], scalar2=None,
                    op0=mybir.AluOpType.is_lt,
                )
                q = ws.tile([128, D_IN], BF16, tag="w_q", bufs=3,
                            name=f"q_{a}")
                nc.vector.tensor_sub(q[:], qp[:], qn[:])
                return q

            def w_emit(a, q):
                """Transpose ternary q and evict into DoubleRow layouts."""
                n_idx, o_off = divmod(a * 128, N_SLICE)
                for g in range(2):
                    wt_ps = tp.tile([128, 8, 128], BF16, tag="xtps", bufs=5,
                                    name=f"wpt_{a}_{g}")
                    for j in range(8):
                        k = g * 8 + j
                        nc.tensor.transpose(
                            wt_ps[:, j, :], q[:, k * 128 : (k + 1) * 128],
                            ident_bf[:],
                        )
                    osl = slice(o_off, o_off + 128)
                    # hi: k16 0..7 -> slot 0, k16 8..15 -> slot 1
                    nc.scalar.copy(
                        w_all[n_idx][:, 0:N_HI, g, osl], wt_ps[:]
                    )
                    if g == 0:
                        # lo: k16 0..3 -> slot 0, k16 4..7 -> slot 1
                        nc.vector.tensor_copy(
                            w_all[n_idx][:, N_HI : N_HI + N_LO, 0, osl],
                            wt_ps[:, 0:4, :],
                        )
                        nc.vector.tensor_copy(
                            w_all[n_idx][:, N_HI : N_HI + N_LO, 1, osl],
                            wt_ps[:, 4:8, :],
                        )

            def so_slice(n):
                """Broadcast scales of slice n across partitions."""
                soT_sb = ws.tile([4, 128], F32R, tag="soT", name=f"soT_{n}")
                t_ps = ac.tile([128, N_SLICE], F32, tag="acc", bufs=3,
                               name=f"sot_ps_{n}")
                nc.tensor.transpose(
                    t_ps[0:4, 0:128].bitcast(F32R),
                    so_col[:, 4 * n : 4 * n + 4],
                    ident_fr[:],
                )
                nc.scalar.copy(soT_sb[:], t_ps[0:4, 0:128])
                bc = ac.tile([128, N_SLICE], F32, tag="acc", bufs=3,
                             name=f"so_bc_{n}")
                for t in range(4):
                    nc.tensor.matmul(
                        bc[:, t * 128 : (t + 1) * 128],
                        sel[:, t * 128 : (t + 1) * 128],
                        soT_sb[:],
                        start=True, stop=True,
                    )
                nc.scalar.copy(so_full[n][:], bc[:])

            def x_prefetch(m):
                x_in = xs.tile([128, D_IN], F32, tag="x_in", bufs=4,
                               name=f"x_in_{m}")
                nc.sync.dma_start(x_in[:], x_d[m * 128 : (m + 1) * 128, :])
                return x_in

            def x_stage(m, x_in=None):
                """Load x row-tile m, bf16, transpose, hi/lo split to fp8."""
                if x_in is None:
                    x_in = x_prefetch(m)
                x_bf = xs.tile([128, D_IN], BF16, tag="x_bf", bufs=2,
                               name=f"x_bf_{m}")

                x_t = xs.tile([128, N_HI + N_LO, 2, 128], F8, tag="x_t",
                              bufs=14, name=f"x_t_{m}")
                for g in range(2):
                    gsl = slice(g * 1024, (g + 1) * 1024)
                    nc.scalar.copy(x_bf[:, gsl], x_in[:, gsl])
                    pt = tp.tile([128, 8, 128], BF16, tag="xtps", bufs=5,
                                 name=f"xpt_{m}_{g}")
                    for j in range(8):
                        k = g * 8 + j
                        nc.tensor.transpose(
                            pt[:, j, :], x_bf[:, k * 128 : (k + 1) * 128],
                            ident_bf[:],
                        )
                    hi = x_t[:, 0:N_HI, g, :]
                    nc.vector.tensor_copy(hi, pt[:])
                    if g == 0:
                        # lo = bf16(x) - hi for k < 1024, into (k, k+512)
                        nc.vector.tensor_tensor(
                            out=x_t[:, N_HI : N_HI + N_LO, 0, :],
                            in0=pt[:, 0:4, :], in1=hi[:, 0:4, :],
                            op=mybir.AluOpType.subtract,
                        )
                        nc.vector.tensor_tensor(
                            out=x_t[:, N_HI : N_HI + N_LO, 1, :],
                            in0=pt[:, 4:8, :], in1=hi[:, 4:8, :],
                            op=mybir.AluOpType.subtract,
                        )
                return x_t

            def mm_group(m, n, x_t):
                """One 12-matmul DoubleRow group + scaled fp16 store."""
                nmm = N_HI + N_LO
                acc = ac.tile([128, N_SLICE], F32, tag="acc", bufs=3,
                              name=f"acc{n}_{m}")
                for i in range(nmm):
                    nc.tensor.matmul(
                        acc[:],
                        x_t[:, i, :, :],
                        w_all[n][:, i, :, :],
                        start=(i == 0),
                        stop=(i == nmm - 1),
                        perf_mode=DR,
                    )
                y_sb = ysp.tile([128, N_SLICE], F16, tag="y_sb", bufs=6,
                                name=f"y_sb{n}_{m}")
                nc.vector.tensor_tensor(
                    out=y_sb[:], in0=acc[:], in1=so_full[n][:],
                    op=mybir.AluOpType.mult,
                )
                nc.sync.dma_start(
                    y_d[m * 128 : (m + 1) * 128,
                        n * N_SLICE : (n + 1) * N_SLICE],
                    y_sb[:],
                )

            # ---- schedule ----
            # Merged pipeline: per step, one W chunk advances (quant 2
            # ahead of its PE transposes), one x tile stages, and up to
            # two matmul groups run as their (x_t, w slice) pair becomes
            # ready. This keeps the DMA (x+w+y) and PE (transposes+mm)
            # both ~80-90% loaded through the fill; phase B then runs
            # PE-bound with the leftover groups drained one per step.
            xts = {}
            qs = {}
            ready_n = set()
            pend = []           # mm groups still to run for m < NPRE
            xin0 = x_prefetch(0)
            xin1 = x_prefetch(1)
            qs[0] = w_quant(0)
            qs[1] = w_quant(1)
            xts[0] = x_stage(0, xin0)
            xts[1] = x_stage(1, xin1)
            xq = list(range(2, NPRE + 2))   # x tiles to stage during fill

            def run_avail(budget):
                ran = 0
                for mn in list(pend):
                    if ran >= budget:
                        break
                    m, n = mn
                    if n in ready_n and m in xts:
                        mm_group(m, n, xts[m])
                        pend.remove(mn)
                        ran += 1
                return ran

            for s_ in range(O_TILES):
                w_emit(s_, qs.pop(s_))
                if s_ + 2 < O_TILES:
                    qs[s_ + 2] = w_quant(s_ + 2)
                if s_ % 4 == 3:
                    so_slice(s_ // 4)
                    n = s_ // 4
                    ready_n.add(n)
                    pend.extend((m, n) for m in range(NPRE))
                for _ in range(2 if s_ < 2 else 1):
                    if xq:
                        mx = xq.pop(0)
                        xts[mx] = x_stage(mx)
                run_avail(2)

            # Phase B: full groups for m>=NPRE plus backlog drain
            for m in range(NPRE, M_TILES):
                if m not in xts:
                    xts[m] = x_stage(m)
                for n in range(N_SLICES):
                    mm_group(m, n, xts[m])
                run_avail(1)
                for mp in list(range(NPRE)):
                    if mp in xts and not any(b[0] == mp for b in pend):
                        del xts[mp]
                if m + 2 < M_TILES and (m + 2) not in xts:
                    xts[m + 2] = x_stage(m + 2)
            while pend:
                run_avail(len(pend))

    nc.compile()
    return nc


_NC_CACHE = None


def _get_nc():
    global _NC_CACHE
    if _NC_CACHE is None:
        _NC_CACHE = _build()
    return _NC_CACHE


def kernel(x: np.ndarray, weight: np.ndarray, _trace: bool = False):
    assert x.shape == (B, S, D_IN) and weight.shape == (D_OUT, D_IN)
    x_flat = np.ascontiguousarray(x.reshape(R, D_IN), dtype=np.float32)
    in_maps = []
    for c in range(NCORES):
        r, col = divmod(c, CGRP)
        in_maps.append(
            {
                "x": np.ascontiguousarray(x_flat[r * R_SH : (r + 1) * R_SH]),
                "w": np.ascontiguousarray(
                    weight[col * O_SH : (col + 1) * O_SH], dtype=np.float32
                ),
            }
        )
    nc = _get_nc()
    res = run_bass_kernel_spmd(
        nc, in_maps, core_ids=list(range(NCORES)), trace=_trace
    )
    y = np.empty((R, D_OUT), dtype=np.float32)
    for c in range(NCORES):
        r, col = divmod(c, CGRP)
        y[r * R_SH : (r + 1) * R_SH, col * O_SH : (col + 1) * O_SH] = (
            res.results[c]["y"]
        )
    out = y.reshape(B, S, D_OUT)
    if _trace:
        return out, res
    return out
